# revision 5
# baseline (speedup 1.0000x reference)
"""GNN message-passing (gather + segment_sum) Trainium2 Bass kernel.

Strategy (node-parallel over destination blocks):
  - Pad node space to 50176 = 8 cores x 49 groups x 128 nodes. Core c owns
    dst nodes [c*6272, (c+1)*6272); no cross-core reduction needed.
  - Host sorts edges by (dst group, src-half) and packs, per core, int16
    gather indices (dma_gather requires int16, so the x table is addressed
    as two halves split at row 32768) plus per-slot group-relative dst
    values (f32, -1 for padding).
  - Device, per 128-node group: dma_gather the edge messages from the
    padded x table (HBM, 256B stride), build a one-hot selection matrix
    B[edge, node] = (dst_rel[edge] == iota[node]) on VectorE, and
    accumulate out_g = sum_chunks B^T @ msgs on TensorE into PSUM (exact
    f32 segment-sum; duplicate dst handled by the matmul reduction).
  - PSUM -> SBUF -> HBM per group; host concatenates core outputs.

Self-contained: hardcodes the problem shapes from the spec.
"""

import math

import numpy as np

import concourse.bass as bass
import concourse.tile as tile
from concourse.bass import _add_dep_helper
from concourse import bacc, mybir
from concourse.alu_op_type import AluOpType
from concourse.bass_utils import run_bass_kernel_spmd

N_NODES = 50000
D_FEAT = 32
N_CORES = 8
G = 128  # dst nodes per group
GROUPS_PER_CORE = 49
N_GROUPS_TOT = N_CORES * GROUPS_PER_CORE  # 392
N_PAD = N_GROUPS_TOT * G  # 50176
LO_ROWS = 32768  # x-table split so gather indices fit int16
ELEM = 64  # f32 per padded x row (256B stride, dma_gather constraint)
CALL = 1024  # max idxs per dma_gather call (SWDGE ring/scratch limit)
MSG_BUFS = 3


def _call_sizes(n_chunks):
    """Split n_chunks*128 idx slots into dma_gather calls of <= CALL idxs."""
    sizes = []
    left = n_chunks * 128
    while left > 0:
        s = min(CALL, left)
        sizes.append(s)
        left -= s
    return sizes


def _prep(x, edge_index):
    """Host-side packing. Returns per-core input maps + schedule constants."""
    src = np.asarray(edge_index[0], dtype=np.int64)
    dst = np.asarray(edge_index[1], dtype=np.int64)
    E = src.shape[0]

    grp = dst >> 7
    half = (src >= LO_ROWS).astype(np.int64)
    order = np.lexsort((half, grp))
    src_s, dst_s, grp_s, half_s = src[order], dst[order], grp[order], half[order]

    key = grp_s * 2 + half_s
    cnt = np.bincount(key, minlength=2 * N_GROUPS_TOT)
    cnt_lo, cnt_hi = cnt[0::2], cnt[1::2]
    L_CH = max(1, math.ceil(int(cnt_lo.max()) / 128))
    H_CH = max(1, math.ceil(int(cnt_hi.max()) / 128))
    C = L_CH + H_CH

    idx_cols = GROUPS_PER_CORE * C * 8  # 16-wrapped idx columns per core
    dr_cols = GROUPS_PER_CORE * C

    slot = np.arange(E) - (np.cumsum(cnt) - cnt)[key]
    core = grp_s // GROUPS_PER_CORE
    g_in = grp_s % GROUPS_PER_CORE

    idx_arr = np.full((N_CORES, 16, idx_cols), -1, np.int16)
    dr_arr = np.full((N_CORES, 128, dr_cols), -1.0, np.float32)

    idxval = (src_s - half_s * LO_ROWS).astype(np.int16)
    icol = g_in * C * 8 + half_s * L_CH * 8 + slot // 16
    idx_arr[core, slot % 16, icol] = idxval
    dcol = g_in * C + half_s * L_CH + slot // 128
    dr_arr[core, slot % 128, dcol] = (dst_s - (grp_s << 7)).astype(np.float32)

    # Per-call valid counts (decode-side ring reservation reads these from a
    # register and they must equal the post-trim index count). Calls whose
    # slots are entirely padding get one guard idx=0 (dst_rel stays -1 ->
    # zero contribution): keeps CoreSim's gather exec and the ucode trim on
    # the nonempty path.
    lo_sizes = _call_sizes(L_CH)
    hi_sizes = _call_sizes(H_CH)
    n_calls = GROUPS_PER_CORE * (len(lo_sizes) + len(hi_sizes))
    cnts = np.zeros((N_CORES, 1, n_calls), np.int32)
    for c in range(N_CORES):
        k = 0
        for g in range(GROUPS_PER_CORE):
            for h, sizes in ((0, lo_sizes), (1, hi_sizes)):
                n_real = int(cnt[(c * GROUPS_PER_CORE + g) * 2 + h])
                off = 0
                for s in sizes:
                    valid = min(max(n_real - off, 0), s)
                    if valid == 0:  # fully-padded call: place guard idx
                        col0 = g * C * 8 + h * L_CH * 8 + off // 16
                        idx_arr[c, 0, col0] = 0
                        valid = 1
                    cnts[c, 0, k] = valid
                    k += 1
                    off += s

    xpad = np.zeros((N_NODES, ELEM), np.float32)
    xpad[:, :D_FEAT] = np.asarray(x, dtype=np.float32)

    iota = np.broadcast_to(
        np.arange(G, dtype=np.float32)[None, :], (128, G)
    ).copy()

    ins = []
    for c in range(N_CORES):
        ins.append(
            {
                "xpad": xpad,
                "idx16": np.tile(idx_arr[c], (8, 1)),
                "dstrel": dr_arr[c],
                "iota": iota,
                "cnts": cnts[c],
            }
        )
    return ins, L_CH, H_CH, idx_cols, dr_cols, n_calls


def _build(reps, L_CH, H_CH, idx_cols, dr_cols, n_calls):
    C = L_CH + H_CH
    nc = bacc.Bacc(
        "TRN2", target_bir_lowering=False, debug=False, num_devices=N_CORES
    )
    f32 = mybir.dt.float32
    xpad = nc.dram_tensor("xpad", [N_NODES, ELEM], f32, kind="ExternalInput")
    idx16 = nc.dram_tensor(
        "idx16", [128, idx_cols], mybir.dt.int16, kind="ExternalInput"
    )
    dstrel = nc.dram_tensor("dstrel", [128, dr_cols], f32, kind="ExternalInput")
    iota = nc.dram_tensor("iota", [128, G], f32, kind="ExternalInput")
    cnts = nc.dram_tensor("cnts", [1, n_calls], mybir.dt.int32, kind="ExternalInput")
    out = nc.dram_tensor(
        "out", [GROUPS_PER_CORE * G, D_FEAT], f32, kind="ExternalOutput"
    )

    lo_sizes = _call_sizes(L_CH)
    hi_sizes = _call_sizes(H_CH)
    x_lo = xpad.ap()[0:LO_ROWS, :]
    x_hi = xpad.ap()[LO_ROWS:N_NODES, :]

    with tile.TileContext(nc) as tc:
        with (
            tc.tile_pool(name="meta", bufs=1) as meta,
            tc.tile_pool(name="msg", bufs=MSG_BUFS) as msgp,
            tc.tile_pool(name="bsel", bufs=4) as bselp,
            tc.tile_pool(name="ps", bufs=2, space="PSUM") as psp,
            tc.tile_pool(name="stage", bufs=2) as stagep,
        ):
            idx_t = meta.tile([128, idx_cols], mybir.dt.int16)
            nc.sync.dma_start(idx_t[:], idx16.ap())
            dr_t = meta.tile([128, dr_cols], f32)
            nc.sync.dma_start(dr_t[:], dstrel.ap())
            iota_t = meta.tile([128, G], f32)
            nc.sync.dma_start(iota_t[:], iota.ap())
            cnt_t = meta.tile([1, n_calls], mybir.dt.int32)
            nc.sync.dma_start(cnt_t[:], cnts.ap())
            cnt_reg = nc.gpsimd.alloc_register("gather_cnt")

            def body(_=None):
                call_k = 0
                prev_gather = None
                for g in range(GROUPS_PER_CORE):
                    msgs = msgp.tile([128, C, ELEM], f32)
                    if g < MSG_BUFS:
                        # virgin SBUF may hold NaN bit patterns; padded slots
                        # must be finite (B row is 0 but 0*NaN = NaN in PE)
                        nc.vector.memset(msgs[:], 0.0)
                    ccol = 0
                    for h, sizes, base_ap in (
                        (0, lo_sizes, x_lo),
                        (1, hi_sizes, x_hi),
                    ):
                        icol = g * C * 8 + h * L_CH * 8
                        for s in sizes:
                            k = s // 128
                            ld = nc.gpsimd.reg_load(
                                cnt_reg, cnt_t[0:1, call_k : call_k + 1]
                            )
                            if prev_gather is not None:
                                # the register is re-used: its load must not
                                # overtake the previous gather's decode
                                _add_dep_helper(
                                    ld.ins, prev_gather.ins, False,
                                    reason="cnt reg WAR on prior gather",
                                )
                            gth = nc.gpsimd.dma_gather(
                                msgs[:, ccol : ccol + k, :],
                                base_ap,
                                idx_t[:, icol : icol + s // 16],
                                s,
                                cnt_reg,
                                ELEM,
                                elem_step=ELEM,
                            )
                            _add_dep_helper(
                                gth.ins, ld.ins, False,
                                reason="num_idxs_reg load before gather",
                            )
                            prev_gather = gth
                            call_k += 1
                            ccol += k
                            icol += s // 16
                    ps = psp.tile([128, D_FEAT], f32)
                    for c in range(C):
                        bt = bselp.tile([128, G], f32)
                        nc.vector.tensor_scalar(
                            bt[:],
                            iota_t[:],
                            dr_t[:, g * C + c : g * C + c + 1],
                            None,
                            AluOpType.is_equal,
                        )
                        nc.tensor.matmul(
                            out=ps[:],
                            lhsT=bt[:],
                            rhs=msgs[:, c, 0:D_FEAT],
                            start=(c == 0),
                            stop=(c == C - 1),
                        )
                    st = stagep.tile([128, D_FEAT], f32)
                    nc.scalar.copy(st[:], ps[:])
                    nc.sync.dma_start(out.ap()[g * G : (g + 1) * G, :], st[:])

            if reps == 1:
                body()
            else:
                with tc.For_i(0, reps) as _i:
                    body(_i)
    nc.compile()
    return nc


_CACHE = {}


def _get_nc(reps, L_CH, H_CH, idx_cols, dr_cols, n_calls):
    key = (reps, L_CH, H_CH, idx_cols, dr_cols, n_calls)
    if key not in _CACHE:
        _CACHE[key] = _build(reps, L_CH, H_CH, idx_cols, dr_cols, n_calls)
    return _CACHE[key]


def run(x, edge_index, reps=1):
    ins, L_CH, H_CH, idx_cols, dr_cols, n_calls = _prep(x, edge_index)
    nc = _get_nc(reps, L_CH, H_CH, idx_cols, dr_cols, n_calls)
    res = run_bass_kernel_spmd(nc, ins, core_ids=list(range(N_CORES)))
    full = np.concatenate([res.results[c]["out"] for c in range(N_CORES)], axis=0)
    return full[:N_NODES]


def kernel(x, edge_index):
    return run(x, edge_index, reps=1)
